# revision 6
# baseline (speedup 1.0000x reference)
"""Trainium2 Bass kernel for batched attention with query-axis softmax.

Reference computation (per example b of 64):
    Q = q @ Wq.T + bq              # [S=1024, Y=128]
    K = q @ Wk.T + bk
    V = q @ Wv.T + bv
    scores = Q @ K.T / sqrt(Y)     # [Sq, Sk]
    attn   = softmax(scores, axis=-2)   # normalize over the QUERY axis
    out    = attn @ V              # [S, Y]
    result = max(out, axis=-2)     # [Y]

Key structural facts exploited here:
  * softmax normalizes over q, which is NOT the contraction axis of attn@V:
    out[q,d] = sum_k U[q,k]/c[k] * V[k,d] with U = exp(scores),
    c[k] = sum_q U[q,k].  So the normalization folds into V's rows:
    out = U @ (V / c).  No SxS division needed.
  * storing scores transposed (scoresT[k,q]) makes c a free-dim row-sum,
    which the ScalarE Exp instruction produces for free via accum_out.
  * outT[d,q] = V'.T-accumulated matmul keeps the final max a free-dim
    reduce_max -> [128,1] per example.

The kernel is ACT-bound: 64 exp instructions of [128,1024] plus their
accumulator reads pace the whole pipeline at ~1.27us/k-tile.  This
version therefore optimizes the startup latency (4-way chunked DMA of
example 0 + chunk-granular projection + early activation-table load),
the tail latency (per-k-tile normalization for the last example's second
half), and thins the DVE instruction stream (grouped reciprocal and
V-scale over 4 k-tiles) so the vector engine never paces the exp stream.

All matmul operands are fp16; accumulation is fp32 in PSUM and the
softmax sums/normalization are fp32.

Sharding: data-parallel over batch, 8 examples per NeuronCore x 8 cores.
"""

import numpy as np
from contextlib import ExitStack

import concourse.bacc as bacc
import concourse.tile as tile
import concourse.mybir as mybir
import concourse.bass_utils as bass_utils

F32 = mybir.dt.float32
BF16 = mybir.dt.float16  # 16-bit matmul dtype: fp16 (11-bit significand)

NCORES = 8
B_PER_CORE = 8
S = 1024          # sequence length
X = 256           # input dim
Y = 128           # head dim
P = 128           # partitions
NH = 2            # 512-column halves of S (psum bank limit)
NKT = S // P      # 8 k-tiles


def emit(ctx, tc, out_d, ins):
    nc = tc.nc
    AF = mybir.ActivationFunctionType
    AX = mybir.AxisListType

    qt_d, w_d, b_d = ins

    wpool = ctx.enter_context(tc.tile_pool(name="w", bufs=1))
    qtp = ctx.enter_context(tc.tile_pool(name="qtp", bufs=4))
    qkp = ctx.enter_context(tc.tile_pool(name="qk", bufs=2))
    up = ctx.enter_context(tc.tile_pool(name="u", bufs=11))
    vrp = ctx.enter_context(tc.tile_pool(name="vr", bufs=3))
    vsp = ctx.enter_context(tc.tile_pool(name="vs", bufs=3))
    crp = ctx.enter_context(tc.tile_pool(name="cr", bufs=6))
    resp = ctx.enter_context(tc.tile_pool(name="res", bufs=1))
    # PSUM budget (8 banks): scores 2x2 + proj 1 + attnV-accum 2 + V 1
    pmm = ctx.enter_context(tc.tile_pool(name="pmm", bufs=2, space="PSUM"))
    pprj = ctx.enter_context(tc.tile_pool(name="pprj", bufs=1, space="PSUM"))
    pout = ctx.enter_context(tc.tile_pool(name="pout", bufs=1, space="PSUM"))
    pvp = ctx.enter_context(tc.tile_pool(name="pv", bufs=1, space="PSUM"))

    # Pull the exp table load off the critical path: a dummy activation at
    # the very top makes walrus place ACT_TABLE_LOAD during the input DMA.
    scr = wpool.tile([P, 1], F32)
    scr2 = wpool.tile([P, 1], F32)
    nc.gpsimd.memset(scr[:], 0)
    nc.scalar.activation(scr2[:], scr[:], AF.Exp)

    # Constants.
    # w: [128, 3*256] bf16 -- wq | wk | wv, each [128, 2*Y] (x-chunk xb at
    #    columns xb*Y..), projection scale folded into wq.
    # b: [128, 2+128] f32 -- bq_scaled | bk | identity (for final transpose)
    w = wpool.tile([P, 7 * Y], BF16)
    nc.gpsimd.dma_start(w[:], w_d[:])
    bqk = wpool.tile([P, 2 + P], F32)
    nc.gpsimd.dma_start(bqk[:], b_d[:])
    wq = w[:, 0 * Y: 2 * Y]
    wk = w[:, 2 * Y: 4 * Y]
    wv = w[:, 4 * Y: 6 * Y]

    def load_qt(b, eng):
        # qT[b] : [256, 1024] -> sbuf [128, 2*1024], x-chunk xb at cols xb*S..
        qt = qtp.tile([P, 2 * S], BF16, tag="qt")
        qv = qt_d[b].rearrange("(xb p) s -> p xb s", p=P)
        eng.dma_start(qt[:].rearrange("p (xb s) -> p xb s", xb=2), qv)
        return qt

    def proj_half(qt, dst, w_sb, bcol, nh):
        # One 512-column half of a Q/K projection: ZT[y, s_half] = W.T @ qT
        pm = pprj.tile([P, 512], F32, tag="pj")
        for xb in range(2):
            nc.tensor.matmul(
                pm[:],
                lhsT=w_sb[:, xb * Y:(xb + 1) * Y],
                rhs=qt[:, xb * S + nh * 512: xb * S + nh * 512 + 512],
                start=(xb == 0),
                stop=(xb == 1),
            )
        # psum -> sbuf with per-partition bias
        nc.vector.tensor_scalar_add(
            dst[:, nh * 512:(nh + 1) * 512], pm[:], bqk[:, bcol:bcol + 1]
        )

    # Rolling per-(example, 4-k-tile-group) state: c accum tile, raw-V
    # block, scaled-V block.
    cg = {}
    vstiles = {}   # (b, g) -> [128, 512] V' tile
    utiles = {}    # (b, kt) -> exp tile

    def front(qt, QT, KT, b, kt):
        """scores -> exp(+colsum) -> V -> V/c for one k-tile."""
        j = kt % 4
        g = kt // 4
        last_ex = (b == B_PER_CORE - 1)
        ps = pmm.tile([P, S], F32, tag="mm")
        with tc.high_priority(offset=40):
            for nh in range(NH):
                nc.tensor.matmul(
                    ps[:, nh * 512:(nh + 1) * 512],
                    lhsT=KT[:, kt * P:(kt + 1) * P],
                    rhs=QT[:, nh * 512: nh * 512 + 512],
                    start=True,
                    stop=True,
                )
            # U = exp(scoresT), c[k] = sum_q U (free accumulation on ACT)
            u = up.tile([P, S], BF16, tag="u")
            if j == 0:
                cg["c"] = crp.tile([P, 4], F32, tag="c", name="c4")
            nc.scalar.activation(u[:], ps[:], AF.Exp,
                                 accum_out=cg["c"][:, j:j + 1])
        utiles[(b, kt)] = u

        # V k-tile directly in [k, d] layout: V[s_tile,:] =
        #   qT_chunk.T @ WvT; bias added during the PSUM drain.
        pv = pvp.tile([P, P], F32, tag="pv")
        for xb in range(2):
            nc.tensor.matmul(
                pv[:],
                lhsT=qt[:, xb * S + kt * P: xb * S + (kt + 1) * P],
                rhs=wv[:, xb * Y:(xb + 1) * Y],
                start=(xb == 0),
                stop=(xb == 1),
            )
        # Drain V out of PSUM right away (frees the single pv bank), adding
        # the bv bias via partition-broadcast, into the group's raw block.
        if j == 0:
            cg["vblk"] = vrp.tile([P, 4 * P], BF16, tag="vb", name="vblk")
        vblk = cg["vblk"]
        nc.vector.tensor_add(vblk[:, j * P:(j + 1) * P], pv[:],
                             w[:, 6 * Y:7 * Y])

        # V'[k, :] = V[k, :] / c[k].  Normally batched per 4-k-tile group
        # (one reciprocal + one broadcast multiply); for the last example's
        # final group done per-k-tile so the tail drains immediately.
        if last_ex and kt >= 4:
            if j == 0:
                vstiles[(b, g)] = vsp.tile([P, 4 * P], BF16, tag="vs", name="vs")
            r1 = crp.tile([P, 1], F32, tag="r1")
            nc.vector.reciprocal(r1[:], cg["c"][:, j:j + 1])
            nc.vector.tensor_scalar_mul(
                vstiles[(b, g)][:, j * P:(j + 1) * P],
                vblk[:, j * P:(j + 1) * P], r1[:])
        elif j == 3:
            r4 = crp.tile([P, 4], F32, tag="r4")
            nc.vector.reciprocal(r4[:], cg["c"][:])
            vs = vsp.tile([P, 4 * P], BF16, tag="vs")
            vstiles[(b, g)] = vs
            nc.vector.tensor_mul(
                vs[:].rearrange("p (g f) -> p g f", g=4),
                vblk[:].rearrange("p (g f) -> p g f", g=4),
                r4[:].unsqueeze(2).broadcast_to((P, 4, P)),
            )

    # Software-pipelined emission over a flat (b, kt) step stream.  The
    # attnV accumulation runs LAG steps behind the scores->exp front so the
    # in-order PE always has the next exp's scores queued ahead of
    # slack-tolerant work (keeps ACT, the bottleneck engine, saturated), and
    # example b+1's DMA + projections are emitted inside example b's k-loop.
    LAG = 5
    steps = [(b, kt) for b in range(B_PER_CORE) for kt in range(NKT)]
    state = {}       # b -> (qt, QT, KT)
    fifo = {}        # step index -> (b, kt)
    po = None

    # Example 0 input in four 512-column chunks across four DMA queues so
    # the first projection starts as early as possible.
    qt0 = qtp.tile([P, 2 * S], BF16, tag="qt")
    qv0 = qt_d[0].rearrange("(xb p) s -> xb p s", p=P)
    # DMA initiators: sync + scalar are HWDGE, gpsimd is SWDGE.
    qengs = [nc.sync, nc.scalar, nc.gpsimd, nc.sync]
    ci = 0
    for h in range(2):
        for xb in range(2):
            qengs[ci].dma_start(
                qt0[:, xb * S + h * 512: xb * S + h * 512 + 512],
                qv0[xb][:, h * 512: h * 512 + 512],
            )
            ci += 1

    # Example 0 projection, consuming chunks in arrival order with
    # per-half drains.
    QT0 = qkp.tile([P, S], BF16, tag="QT")
    KT0 = qkp.tile([P, S], BF16, tag="KT")
    pmQ = pmm.tile([P, S], F32, tag="mm")
    pmK = pmm.tile([P, S], F32, tag="mm")
    with tc.high_priority():
        for h in range(2):
            for xb in range(2):
                for pm, w_sb in ((pmQ, wq), (pmK, wk)):
                    nc.tensor.matmul(
                        pm[:, h * 512: h * 512 + 512],
                        lhsT=w_sb[:, xb * Y:(xb + 1) * Y],
                        rhs=qt0[:, xb * S + h * 512: xb * S + h * 512 + 512],
                        start=(xb == 0),
                        stop=(xb == 1),
                    )
            nc.vector.tensor_scalar_add(
                QT0[:, h * 512: h * 512 + 512],
                pmQ[:, h * 512: h * 512 + 512], bqk[:, 0:1])
            nc.vector.tensor_scalar_add(
                KT0[:, h * 512: h * 512 + 512],
                pmK[:, h * 512: h * 512 + 512], bqk[:, 1:2])
    state[0] = (qt0, QT0, KT0)

    res_all = resp.tile([P, B_PER_CORE], F32, tag="res")

    def drain(i):
        nonlocal po
        b, kt = fifo.pop(i)
        u = utiles.pop((b, kt))
        g, j = kt // 4, kt % 4
        vs = vstiles[(b, g)][:, j * P:(j + 1) * P]
        if kt == 0:
            po = pout.tile([P, S], F32, tag="out")
        # outT[d, q] += V'.T @ U   (contract k)
        for nh in range(NH):
            nc.tensor.matmul(
                po[:, nh * 512:(nh + 1) * 512],
                lhsT=vs,
                rhs=u[:, nh * 512: nh * 512 + 512],
                start=(kt == 0),
                stop=(kt == NKT - 1),
            )
        if kt == NKT - 1:
            nc.vector.reduce_max(res_all[:, b:b + 1], po[:], axis=AX.X)
            vstiles.pop((b, 0), None)
            vstiles.pop((b, 1), None)

    qtiles = {0: qt0}
    qtiles[1] = load_qt(1, nc.sync)
    qtiles[2] = load_qt(2, nc.sync)

    for i, (b, kt) in enumerate(steps):
        qt, QT, KT = state[b]
        if kt == 0 and b + 1 < B_PER_CORE:
            state[b + 1] = (qtiles[b + 1],)
        if kt == 1 and b + 3 < B_PER_CORE:
            qtiles[b + 3] = load_qt(b + 3, nc.sync)
        if kt == 2 and b + 1 < B_PER_CORE:
            # allocate next example's projection outputs; halves fill in
            # one per step over kt=2..5
            QT_n = qkp.tile([P, S], BF16, tag="QT")
            KT_n = qkp.tile([P, S], BF16, tag="KT")
            state[b + 1] = (state[b + 1][0], QT_n, KT_n)
        if 2 <= kt <= 5 and b + 1 < B_PER_CORE:
            qt_n, QT_n, KT_n = state[b + 1]
            w_sb, bcol, dst = ((wq, 0, QT_n), (wk, 1, KT_n))[(kt - 2) // 2]
            proj_half(qt_n, dst, w_sb, bcol, (kt - 2) % 2)
        front(qt, QT, KT, b, kt)
        fifo[i] = (b, kt)
        target = i - LAG
        if b == B_PER_CORE - 1:
            if kt >= 4:
                target = i - 1       # tail: drain as soon as vs exists
            elif kt >= 3:
                target = i - LAG + (kt - 2)
        while fifo and min(fifo) <= target:
            drain(min(fifo))
    for i in sorted(fifo):
        drain(i)

    # Transpose the collected [128(d), 8(b)] results to [8, 128] on the PE
    # so the single output DMA is 8 dense 512B rows instead of 128 scattered
    # 4B descriptors (which hogs the DMA queue for ~7us).
    pt = pvp.tile([P, P], F32, tag="pv")
    nc.tensor.transpose(pt[0:B_PER_CORE, :], res_all[:], bqk[:, 2:2 + P])
    res_t = resp.tile([B_PER_CORE, P], F32, tag="rest")
    nc.vector.tensor_copy(res_t[:], pt[0:B_PER_CORE, :])
    nc.sync.dma_start(out_d[:], res_t[:])


def build_program():
    nc = bacc.Bacc(
        "TRN2",
        target_bir_lowering=False,
        debug=False,
        enable_asserts=False,
    )
    qt = nc.dram_tensor("qt", [B_PER_CORE, X, S], BF16, kind="ExternalInput").ap()
    w = nc.dram_tensor("w", [P, 7 * Y], BF16, kind="ExternalInput").ap()
    b = nc.dram_tensor("b", [P, 2 + P], F32, kind="ExternalInput").ap()
    out = nc.dram_tensor("out", [B_PER_CORE, Y], F32, kind="ExternalOutput").ap()

    ins = (qt, w, b)
    with tile.TileContext(nc) as tc:
        with ExitStack() as ctx:
            emit(ctx, tc, out, ins)
    nc.compile()
    return nc


_NC_CACHE = None


def _get_program():
    global _NC_CACHE
    if _NC_CACHE is None:
        _NC_CACHE = build_program()
    return _NC_CACHE


def prep_inputs(q, Wq, bq, Wk, bk, Wv, bv):
    """Host-side marshalling: transpose q, pack weights, fold softmax scale."""
    q = np.asarray(q, dtype=np.float32)
    scale = np.float32(1.0 / np.sqrt(Y))
    f16 = np.float16

    qT = np.ascontiguousarray(q.transpose(0, 2, 1)).astype(f16)  # [B, X, S]

    def pack(w):  # [Y, X] torch layout -> [128, 2*Y]: chunk xb at cols xb*Y..
        wt = np.asarray(w, dtype=np.float32).T  # [X, Y]
        return np.concatenate([wt[0:P], wt[P:2 * P]], axis=1)

    w_all = np.concatenate(
        [pack(Wq) * scale, pack(Wk), pack(Wv),
         np.tile(np.asarray(bv, np.float32).reshape(1, Y), (P, 1))], axis=1
    ).astype(f16)
    b_all = np.concatenate(
        [np.stack([np.asarray(bq, np.float32) * scale,
                   np.asarray(bk, np.float32)], axis=1),
         np.eye(P, dtype=np.float32)], axis=1
    ).astype(np.float32)
    feeds = {
        "w": np.ascontiguousarray(w_all),
        "b": np.ascontiguousarray(b_all),
    }
    return qT, feeds


def kernel(q, Wq, bq, Wk, bk, Wv, bv, _trace=False):
    qT, feeds = prep_inputs(q, Wq, bq, Wk, bk, Wv, bv)
    nc = _get_program()
    in_maps = [
        {"qt": qT[c * B_PER_CORE:(c + 1) * B_PER_CORE], **feeds}
        for c in range(NCORES)
    ]
    kw = {}
    if _trace:
        kw = dict(trace=True)
    res = bass_utils.run_bass_kernel_spmd(
        nc, in_maps, core_ids=list(range(NCORES)), **kw
    )
    out = np.concatenate([r["out"] for r in res.results], axis=0)
    if _trace:
        return out, res
    return out
